# revision 51
# baseline (speedup 1.0000x reference)
"""Trainium2 Bass kernel for the NodeEdge GNN message-passing module.

Computes  out[b,n,h] = sum_e (w*inci + b)[n,e] * relu(inputs @ W_xes + b_xes)[b,e,h]
with B=16, N=2048, E=8192, DIM=64, DH=32.

Strategy: shard the edge (contraction) dimension E across the 8 NeuronCores
(EC=1024 edges per core). Each core:
  - computes xe = relu(inputs[:, e_shard, :] @ W_xes) for its edge shard
    in [e, (b,h)] layout via small PE matmuls,
  - forms A^T chunks (w * inci, transposed so e is the partition axis;
    the transpose itself is done on the host as layout prep),
  - runs the big matmul  out_partial[(b,h), n] = xe^T @ A^T  accumulating
    in f32 PSUM.
Partial outputs (one per core, bf16) are summed on the host in f32.

All matmul operands travel as bf16 (full PE rate, half the HBM bytes of
f32). inci ships as uint8 and is multiplied into the w chunks in place
with one mixed-dtype DVE tensor_tensor. Inputs are repacked k-major on
the host so each xe chunk depends on a single 256 KiB DMA, and all DRAM
rows are >= 4 KiB so the HWDGE descriptor rate doesn't cap queue
bandwidth. The in-order sync queue orders inputs ahead of w chunks; the
incidence stream rides the gpsimd queue concurrently.
"""

from contextlib import ExitStack

import ml_dtypes
import numpy as np

import concourse.bass as bass
import concourse.mybir as mybir
import concourse.tile as tile
from concourse import bacc
from concourse.bass_utils import run_bass_kernel_spmd

B, N, E, DIM = 16, 2048, 8192, 64
DH = DIM // 2              # 32
NCORES = 8
EC = E // NCORES           # 1024 edges per core
KC = EC // 128             # 8 e-chunks of 128
KP = KC // 2               # 4 pair tiles (two e-chunks per DMA)
BH = B * DH                # 512 (flattened (b, h) output dim)
NB = N // 512              # 4 column blocks of the big matmul
NJ = B // 2                # 8 lhsT blocks per e-chunk (two batch rows each)

F32 = mybir.dt.float32
BF16 = mybir.dt.bfloat16
U8 = mybir.dt.uint8

_PROGRAMS: dict = {}


def _build_program(with_bxes: bool, with_b: bool):
    nc = bacc.Bacc(
        "TRN2", target_bir_lowering=False, debug=False, enable_asserts=False
    )

    # k-major inputs: tile k holds NJ lhsT blocks of [128 (2b,d), 128 e]
    # side by side, so xe chunk k depends on a single 256 KiB DMA.
    inp_t = nc.dram_tensor("inp_t", [KC, 128, EC], BF16, kind="ExternalInput").ap()
    wq = nc.dram_tensor("wq", [KC, 128, N], BF16, kind="ExternalInput").ap()
    iq = nc.dram_tensor("iq", [KP, 128, 2 * N], U8, kind="ExternalInput").ap()
    wx = nc.dram_tensor("wx", [128, 2 * DH], BF16, kind="ExternalInput").ap()
    bxr = (
        nc.dram_tensor("bxr", [128, BH], F32, kind="ExternalInput").ap()
        if with_bxes
        else None
    )
    bq = (
        nc.dram_tensor("bq", [KC, 128, N], BF16, kind="ExternalInput").ap()
        if with_b
        else None
    )
    outp = nc.dram_tensor("outp", [BH, N], BF16, kind="ExternalOutput").ap()

    with tile.TileContext(nc) as tc, ExitStack() as ctx:
        inp_pool = ctx.enter_context(tc.tile_pool(name="inp", bufs=KC))
        wx_pool = ctx.enter_context(tc.tile_pool(name="wx", bufs=1))
        xe_pool = ctx.enter_context(tc.tile_pool(name="xe", bufs=KC))
        a_pool = ctx.enter_context(tc.tile_pool(name="a", bufs=KC))
        i_pool = ctx.enter_context(tc.tile_pool(name="i", bufs=KP))
        out_pool = ctx.enter_context(tc.tile_pool(name="o", bufs=4))
        ps_pool = ctx.enter_context(tc.tile_pool(name="ps", bufs=8, space="PSUM"))

        # Block-diagonal xes weight: rows 0-63 map the even batch row to
        # output cols 0-31, rows 64-127 map the odd batch row to cols
        # 32-63, so one K=128 matmul computes xe for both packed batch
        # rows of an input tile at once.
        wx_tile = wx_pool.tile([128, 2 * DH], BF16)
        nc.sync.dma_start(wx_tile[:], wx[:])

        # PE warm-up: the HAM clock gate holds the PE at 1.2 GHz until it
        # sees ~3.4us of sustained activity, and the first real matmul
        # can't start until the first input tile lands (~11us in). A
        # bridge of matmuls on a zeroed tile keeps the PE busy from ~5us
        # so the real work starts at the full 2.4 GHz.
        warm_src = wx_pool.tile([128, 512], BF16, tag="warm")
        nc.vector.memset(warm_src[:], 0.0)
        warm_ps = ps_pool.tile([128, 512], F32, tag="ps", name="warm_ps")
        for _ in range(15):
            nc.tensor.matmul(
                warm_ps[:], warm_src[:, 0:128], warm_src[:],
                start=True, stop=True,
            )

        # HBM is the binding resource and the inputs head the critical
        # path: stream them on both HWDGE queues (sync evens + scalar
        # odds) with almost nothing competing. The first incidence pair
        # leads the sync queue (it gates the first mask-multiply, which
        # gates the big matmul); the remaining pairs queue behind the
        # even inputs so they never steal bandwidth from them.
        iu_tiles = [
            i_pool.tile([128, 2 * N], U8, tag="iu", name=f"iu_{kk}", bufs=KP)
            for kk in range(KP)
        ]
        nc.sync.dma_start(iu_tiles[0][:], iq[0])

        a_tiles = [
            a_pool.tile([128, N], BF16, tag="a", bufs=KC, name=f"a_{k}")
            for k in range(KC)
        ]

        inp_tiles = []
        for k in range(KC):
            t = inp_pool.tile([128, EC], BF16, name=f"inp_{k}")
            eng = nc.sync if k % 2 == 0 else nc.scalar
            eng.dma_start(t[:], inp_t[k])
            inp_tiles.append(t)

        for kk in range(1, KP):
            nc.sync.dma_start(iu_tiles[kk][:], iq[kk])



        bx_tile = None
        if with_bxes:
            bx_tile = wx_pool.tile([128, BH], F32, tag="bx")
            nc.sync.dma_start(bx_tile[:], bxr[:])

        # ---- xe = relu(inputs @ W_xes) in [e, (b,h)] layout ----
        # Each wq chunk is issued from the scalar engine right after
        # relu k: the in-order engine releases the w stream a chunk
        # behind the input stream, so the w traffic never steals HBM
        # bandwidth from the inputs it depends on.
        xe_tiles = []
        for k in range(KC):
            ps = ps_pool.tile([128, BH], F32, tag="ps")
            for j in range(NJ):
                lhsT = inp_tiles[k][:, j * 128 : (j + 1) * 128]
                nc.tensor.matmul(
                    ps[:, j * 2 * DH : (j + 1) * 2 * DH],
                    lhsT,
                    wx_tile[:],
                    start=True,
                    stop=True,
                )
            xt = xe_pool.tile([128, BH], BF16)
            if with_bxes:
                nc.vector.tensor_tensor(
                    xt[:], ps[:], bx_tile[:], op=mybir.AluOpType.add
                )
                nc.scalar.activation(
                    xt[:], xt[:], mybir.ActivationFunctionType.Relu
                )
            else:
                nc.scalar.activation(
                    xt[:], ps[:], mybir.ActivationFunctionType.Relu
                )
            xe_tiles.append(xt)
            nc.scalar.dma_start(a_tiles[k][:], wq[k])

        # ---- A^T chunks: a = w * inci (single mixed-dtype pass) ----
        for k in range(KC):
            nc.vector.tensor_tensor(
                a_tiles[k][:], a_tiles[k][:],
                iu_tiles[k // 2][:, (k % 2) * N : (k % 2 + 1) * N],
                op=mybir.AluOpType.mult,
            )
            if with_b:
                bt = i_pool.tile([128, N], BF16, tag="bt", bufs=2)
                nc.sync.dma_start(bt[:], bq[k])
                nc.vector.tensor_tensor(
                    a_tiles[k][:], a_tiles[k][:], bt[:],
                    op=mybir.AluOpType.add,
                )

        # ---- big matmul: out[(b,h), n] += xe^T @ A^T, bf16, f32 accum ----
        # stores pack two nb blocks into one [128,1024] tile so the DRAM
        # write rows stay at 2 KiB
        def store_pair(pair, half, pstiles, nbp, dma_engine):
            bh = 2 * pair + half
            ot = out_pool.tile(
                [128, 1024], BF16, tag="o", name=f"ot_{pair}_{half}_{nbp}"
            )
            for i in range(2):
                nb = 2 * nbp + i
                sl = ot[:, i * 512 : (i + 1) * 512]
                if i == 0:
                    nc.scalar.activation(
                        sl, pstiles[half][nb][:],
                        mybir.ActivationFunctionType.Identity,
                    )
                else:
                    nc.vector.tensor_copy(sl, pstiles[half][nb][:])
            dma_engine.dma_start(
                outp[bh * 128 : (bh + 1) * 128, nbp * 1024 : (nbp + 1) * 1024],
                ot[:],
            )

        for pair in range(BH // 256):  # two (b,h) 128-chunks at a time
            pstiles = [
                [
                    ps_pool.tile(
                        [128, 512], F32, tag="ps", name=f"bps_{pair}_{h2}_{nb}"
                    )
                    for nb in range(NB)
                ]
                for h2 in range(2)
            ]
            if pair == 0:
                # arrival-paced: walk k outermost so each chunk is used
                # as soon as its A^T tile is ready
                for k in range(KC):
                    for half in range(2):
                        bh = 2 * pair + half
                        lhsT = xe_tiles[k][:, bh * 128 : (bh + 1) * 128]
                        for nb in range(NB):
                            nc.tensor.matmul(
                                pstiles[half][nb][:],
                                lhsT,
                                a_tiles[k][:, nb * 512 : (nb + 1) * 512],
                                start=(k == 0),
                                stop=(k == KC - 1),
                            )
                for half in range(2):
                    for nbp in range(NB // 2):
                        store_pair(0, half, pstiles, nbp, nc.gpsimd)
            else:
                # all data resident: close each nb group after its 8 MMs
                # so stores pipeline with the remaining matmuls
                for half in range(2):
                    bh = 2 * pair + half
                    for nb in range(NB):
                        for k in range(KC):
                            nc.tensor.matmul(
                                pstiles[half][nb][:],
                                xe_tiles[k][:, bh * 128 : (bh + 1) * 128],
                                a_tiles[k][:, nb * 512 : (nb + 1) * 512],
                                start=(k == 0),
                                stop=(k == KC - 1),
                            )
                        if nb % 2 == 1:
                            store_pair(
                                pair, half, pstiles, nb // 2,
                                nc.scalar if half == 0 else nc.sync,
                            )

    nc.compile()
    return nc


def _get_program(with_bxes: bool, with_b: bool):
    key = (with_bxes, with_b)
    if key not in _PROGRAMS:
        _PROGRAMS[key] = _build_program(with_bxes, with_b)
    return _PROGRAMS[key]


def _prepare_in_maps(inputs, W_xes, b_xes, inci, w, b, with_bxes, with_b):
    bf16 = ml_dtypes.bfloat16
    inputs = np.asarray(inputs, dtype=np.float32)
    W_xes = np.asarray(W_xes, dtype=np.float32)
    b_xes = np.asarray(b_xes, dtype=np.float32)
    w = np.asarray(w, dtype=np.float32)
    b = np.asarray(b, dtype=np.float32)
    inci_u8 = np.asarray(inci).astype(np.uint8)

    wx_dup = np.zeros((128, 2 * DH), dtype=bf16)
    wx_dup[0:DIM, 0:DH] = W_xes.astype(bf16)
    wx_dup[DIM : 2 * DIM, DH : 2 * DH] = W_xes.astype(bf16)
    bxr = np.ascontiguousarray(
        np.broadcast_to(np.tile(b_xes, B)[None, :], (128, BH))
    ) if with_bxes else None

    in_maps = []
    for c in range(NCORES):
        sl = slice(c * EC, (c + 1) * EC)
        # k-major inputs: [j, b2, k, e, d] -> [k, (b2,d), j, e]
        t = inputs[:, sl, :].reshape(NJ, 2, KC, 128, DIM)
        t = np.ascontiguousarray(t.transpose(2, 1, 4, 0, 3)).reshape(
            KC, 128, EC
        ).astype(bf16)
        wq_ = np.ascontiguousarray(w[:, sl].T).reshape(KC, 128, N).astype(bf16)
        iq_ = np.ascontiguousarray(
            inci_u8[:, sl].T.reshape(KP, 2, 128, N).transpose(0, 2, 1, 3)
        ).reshape(KP, 128, 2 * N)
        m = {"inp_t": t, "wq": wq_, "iq": iq_, "wx": wx_dup}
        if with_bxes:
            m["bxr"] = bxr
        if with_b:
            m["bq"] = np.ascontiguousarray(b[:, sl].T).reshape(
                KC, 128, N
            ).astype(bf16)
        in_maps.append(m)
    return in_maps


def _run(inputs, W_xes, b_xes, inci, w, b, **run_kwargs):
    with_bxes = bool(np.any(np.asarray(b_xes)))
    with_b = bool(np.any(np.asarray(b)))
    nc = _get_program(with_bxes, with_b)
    in_maps = _prepare_in_maps(inputs, W_xes, b_xes, inci, w, b, with_bxes, with_b)
    res = run_bass_kernel_spmd(
        nc, in_maps, core_ids=list(range(NCORES)), **run_kwargs
    )
    parts = np.stack(
        [np.asarray(r["outp"], dtype=np.float32) for r in res.results]
    )  # [8, BH, N] f32
    out = parts.sum(axis=0)  # [BH, N]
    out = out.reshape(B, DH, N).transpose(0, 2, 1)  # [B, N, DH]
    return np.ascontiguousarray(out.astype(np.float32)), res


def kernel(inputs, W_xes, b_xes, inci, w, b):
    out, _ = _run(inputs, W_xes, b_xes, inci, w, b)
    return out


# revision 52
# speedup vs baseline: 1.0438x; 1.0438x over previous
"""Trainium2 Bass kernel for the NodeEdge GNN message-passing module.

Computes  out[b,n,h] = sum_e (w*inci + b)[n,e] * relu(inputs @ W_xes + b_xes)[b,e,h]
with B=16, N=2048, E=8192, DIM=64, DH=32.

Strategy: shard the edge (contraction) dimension E across the 8 NeuronCores
(EC=1024 edges per core). Each core:
  - computes xe = relu(inputs[:, e_shard, :] @ W_xes) for its edge shard
    in [e, (b,h)] layout via small PE matmuls,
  - forms A^T chunks (w * inci, transposed so e is the partition axis;
    the transpose itself is done on the host as layout prep),
  - runs the big matmul  out_partial[(b,h), n] = xe^T @ A^T  accumulating
    in f32 PSUM.
Partial outputs (one per core, bf16) are summed on the host in f32.

All matmul operands travel as bf16 (full PE rate, half the HBM bytes of
f32). inci ships as uint8 and is multiplied into the w chunks in place
with one mixed-dtype DVE tensor_tensor. Inputs are repacked k-major on
the host so each xe chunk depends on a single 256 KiB DMA, and all DRAM
rows are >= 4 KiB so the HWDGE descriptor rate doesn't cap queue
bandwidth. The in-order sync queue orders inputs ahead of w chunks; the
incidence stream rides the gpsimd queue concurrently.
"""

from contextlib import ExitStack

import ml_dtypes
import numpy as np

import concourse.bass as bass
import concourse.mybir as mybir
import concourse.tile as tile
from concourse import bacc
from concourse.bass_utils import run_bass_kernel_spmd

B, N, E, DIM = 16, 2048, 8192, 64
DH = DIM // 2              # 32
NCORES = 8
EC = E // NCORES           # 1024 edges per core
KC = EC // 128             # 8 e-chunks of 128
KP = KC // 2               # 4 pair tiles (two e-chunks per DMA)
BH = B * DH                # 512 (flattened (b, h) output dim)
NB = N // 512              # 4 column blocks of the big matmul
NJ = B // 2                # 8 lhsT blocks per e-chunk (two batch rows each)

F32 = mybir.dt.float32
BF16 = mybir.dt.bfloat16
U8 = mybir.dt.uint8

_PROGRAMS: dict = {}


def _build_program(with_bxes: bool, with_b: bool):
    nc = bacc.Bacc(
        "TRN2", target_bir_lowering=False, debug=False, enable_asserts=False
    )

    # k-major inputs: tile k holds NJ lhsT blocks of [128 (2b,d), 128 e]
    # side by side, so xe chunk k depends on a single 256 KiB DMA.
    inp_t = nc.dram_tensor("inp_t", [KC, 128, EC], BF16, kind="ExternalInput").ap()
    wq = nc.dram_tensor("wq", [KC, 128, N], BF16, kind="ExternalInput").ap()
    iq = nc.dram_tensor("iq", [KP, 128, 2 * N], U8, kind="ExternalInput").ap()
    wx = nc.dram_tensor("wx", [128, 2 * DH], BF16, kind="ExternalInput").ap()
    bxr = (
        nc.dram_tensor("bxr", [128, BH], F32, kind="ExternalInput").ap()
        if with_bxes
        else None
    )
    bq = (
        nc.dram_tensor("bq", [KC, 128, N], BF16, kind="ExternalInput").ap()
        if with_b
        else None
    )
    outp = nc.dram_tensor("outp", [BH, N], BF16, kind="ExternalOutput").ap()

    with tile.TileContext(nc) as tc, ExitStack() as ctx:
        inp_pool = ctx.enter_context(tc.tile_pool(name="inp", bufs=KC))
        wx_pool = ctx.enter_context(tc.tile_pool(name="wx", bufs=1))
        xe_pool = ctx.enter_context(tc.tile_pool(name="xe", bufs=KC))
        a_pool = ctx.enter_context(tc.tile_pool(name="a", bufs=KC))
        i_pool = ctx.enter_context(tc.tile_pool(name="i", bufs=KP))
        out_pool = ctx.enter_context(tc.tile_pool(name="o", bufs=4))
        ps_pool = ctx.enter_context(tc.tile_pool(name="ps", bufs=8, space="PSUM"))

        # Block-diagonal xes weight: rows 0-63 map the even batch row to
        # output cols 0-31, rows 64-127 map the odd batch row to cols
        # 32-63, so one K=128 matmul computes xe for both packed batch
        # rows of an input tile at once.
        wx_tile = wx_pool.tile([128, 2 * DH], BF16)
        nc.sync.dma_start(wx_tile[:], wx[:])

        # HBM is the binding resource and the inputs head the critical
        # path: stream them on both HWDGE queues (sync evens + scalar
        # odds) with almost nothing competing. The first incidence pair
        # leads the sync queue (it gates the first mask-multiply, which
        # gates the big matmul); the remaining pairs queue behind the
        # even inputs so they never steal bandwidth from them.
        iu_tiles = [
            i_pool.tile([128, 2 * N], U8, tag="iu", name=f"iu_{kk}", bufs=KP)
            for kk in range(KP)
        ]
        nc.sync.dma_start(iu_tiles[0][:], iq[0])

        a_tiles = [
            a_pool.tile([128, N], BF16, tag="a", bufs=KC, name=f"a_{k}")
            for k in range(KC)
        ]

        inp_tiles = []
        for k in range(KC):
            t = inp_pool.tile([128, EC], BF16, name=f"inp_{k}")
            eng = nc.sync if k % 2 == 0 else nc.scalar
            eng.dma_start(t[:], inp_t[k])
            inp_tiles.append(t)

        for kk in range(1, KP):
            nc.sync.dma_start(iu_tiles[kk][:], iq[kk])



        bx_tile = None
        if with_bxes:
            bx_tile = wx_pool.tile([128, BH], F32, tag="bx")
            nc.sync.dma_start(bx_tile[:], bxr[:])

        # ---- xe = relu(inputs @ W_xes) in [e, (b,h)] layout ----
        # Each wq chunk is issued from the scalar engine right after
        # relu k: the in-order engine releases the w stream a chunk
        # behind the input stream, so the w traffic never steals HBM
        # bandwidth from the inputs it depends on.
        xe_tiles = []
        for k in range(KC):
            ps = ps_pool.tile([128, BH], F32, tag="ps")
            for j in range(NJ):
                lhsT = inp_tiles[k][:, j * 128 : (j + 1) * 128]
                nc.tensor.matmul(
                    ps[:, j * 2 * DH : (j + 1) * 2 * DH],
                    lhsT,
                    wx_tile[:],
                    start=True,
                    stop=True,
                )
            xt = xe_pool.tile([128, BH], BF16)
            if with_bxes:
                nc.vector.tensor_tensor(
                    xt[:], ps[:], bx_tile[:], op=mybir.AluOpType.add
                )
                nc.scalar.activation(
                    xt[:], xt[:], mybir.ActivationFunctionType.Relu
                )
            else:
                nc.scalar.activation(
                    xt[:], ps[:], mybir.ActivationFunctionType.Relu
                )
            xe_tiles.append(xt)
            nc.scalar.dma_start(a_tiles[k][:], wq[k])

        # ---- A^T chunks: a = w * inci (single mixed-dtype pass) ----
        for k in range(KC):
            nc.vector.tensor_tensor(
                a_tiles[k][:], a_tiles[k][:],
                iu_tiles[k // 2][:, (k % 2) * N : (k % 2 + 1) * N],
                op=mybir.AluOpType.mult,
            )
            if with_b:
                bt = i_pool.tile([128, N], BF16, tag="bt", bufs=2)
                nc.sync.dma_start(bt[:], bq[k])
                nc.vector.tensor_tensor(
                    a_tiles[k][:], a_tiles[k][:], bt[:],
                    op=mybir.AluOpType.add,
                )

        # ---- big matmul: out[(b,h), n] += xe^T @ A^T, bf16, f32 accum ----
        # stores pack two nb blocks into one [128,1024] tile so the DRAM
        # write rows stay at 2 KiB
        def store_pair(pair, half, pstiles, nbp, dma_engine):
            bh = 2 * pair + half
            ot = out_pool.tile(
                [128, 1024], BF16, tag="o", name=f"ot_{pair}_{half}_{nbp}"
            )
            for i in range(2):
                nb = 2 * nbp + i
                sl = ot[:, i * 512 : (i + 1) * 512]
                if i == 0:
                    nc.scalar.activation(
                        sl, pstiles[half][nb][:],
                        mybir.ActivationFunctionType.Identity,
                    )
                else:
                    nc.vector.tensor_copy(sl, pstiles[half][nb][:])
            dma_engine.dma_start(
                outp[bh * 128 : (bh + 1) * 128, nbp * 1024 : (nbp + 1) * 1024],
                ot[:],
            )

        for pair in range(BH // 256):  # two (b,h) 128-chunks at a time
            pstiles = [
                [
                    ps_pool.tile(
                        [128, 512], F32, tag="ps", name=f"bps_{pair}_{h2}_{nb}"
                    )
                    for nb in range(NB)
                ]
                for h2 in range(2)
            ]
            if pair == 0:
                # arrival-paced: walk k outermost so each chunk is used
                # as soon as its A^T tile is ready
                for k in range(KC):
                    for half in range(2):
                        bh = 2 * pair + half
                        lhsT = xe_tiles[k][:, bh * 128 : (bh + 1) * 128]
                        for nb in range(NB):
                            nc.tensor.matmul(
                                pstiles[half][nb][:],
                                lhsT,
                                a_tiles[k][:, nb * 512 : (nb + 1) * 512],
                                start=(k == 0),
                                stop=(k == KC - 1),
                            )
                for half in range(2):
                    for nbp in range(NB // 2):
                        store_pair(0, half, pstiles, nbp, nc.gpsimd)
            else:
                # all data resident: close each nb group after its 8 MMs
                # so stores pipeline with the remaining matmuls
                for half in range(2):
                    bh = 2 * pair + half
                    for nb in range(NB):
                        for k in range(KC):
                            nc.tensor.matmul(
                                pstiles[half][nb][:],
                                xe_tiles[k][:, bh * 128 : (bh + 1) * 128],
                                a_tiles[k][:, nb * 512 : (nb + 1) * 512],
                                start=(k == 0),
                                stop=(k == KC - 1),
                            )
                        if nb % 2 == 1:
                            store_pair(
                                pair, half, pstiles, nb // 2,
                                nc.scalar if half == 0 else nc.sync,
                            )

    nc.compile()
    return nc


def _get_program(with_bxes: bool, with_b: bool):
    key = (with_bxes, with_b)
    if key not in _PROGRAMS:
        _PROGRAMS[key] = _build_program(with_bxes, with_b)
    return _PROGRAMS[key]


def _prepare_in_maps(inputs, W_xes, b_xes, inci, w, b, with_bxes, with_b):
    bf16 = ml_dtypes.bfloat16
    inputs = np.asarray(inputs, dtype=np.float32)
    W_xes = np.asarray(W_xes, dtype=np.float32)
    b_xes = np.asarray(b_xes, dtype=np.float32)
    w = np.asarray(w, dtype=np.float32)
    b = np.asarray(b, dtype=np.float32)
    inci_u8 = np.asarray(inci).astype(np.uint8)

    wx_dup = np.zeros((128, 2 * DH), dtype=bf16)
    wx_dup[0:DIM, 0:DH] = W_xes.astype(bf16)
    wx_dup[DIM : 2 * DIM, DH : 2 * DH] = W_xes.astype(bf16)
    bxr = np.ascontiguousarray(
        np.broadcast_to(np.tile(b_xes, B)[None, :], (128, BH))
    ) if with_bxes else None

    in_maps = []
    for c in range(NCORES):
        sl = slice(c * EC, (c + 1) * EC)
        # k-major inputs: [j, b2, k, e, d] -> [k, (b2,d), j, e]
        t = inputs[:, sl, :].reshape(NJ, 2, KC, 128, DIM)
        t = np.ascontiguousarray(t.transpose(2, 1, 4, 0, 3)).reshape(
            KC, 128, EC
        ).astype(bf16)
        wq_ = np.ascontiguousarray(w[:, sl].T).reshape(KC, 128, N).astype(bf16)
        iq_ = np.ascontiguousarray(
            inci_u8[:, sl].T.reshape(KP, 2, 128, N).transpose(0, 2, 1, 3)
        ).reshape(KP, 128, 2 * N)
        m = {"inp_t": t, "wq": wq_, "iq": iq_, "wx": wx_dup}
        if with_bxes:
            m["bxr"] = bxr
        if with_b:
            m["bq"] = np.ascontiguousarray(b[:, sl].T).reshape(
                KC, 128, N
            ).astype(bf16)
        in_maps.append(m)
    return in_maps


def _run(inputs, W_xes, b_xes, inci, w, b, **run_kwargs):
    with_bxes = bool(np.any(np.asarray(b_xes)))
    with_b = bool(np.any(np.asarray(b)))
    nc = _get_program(with_bxes, with_b)
    in_maps = _prepare_in_maps(inputs, W_xes, b_xes, inci, w, b, with_bxes, with_b)
    res = run_bass_kernel_spmd(
        nc, in_maps, core_ids=list(range(NCORES)), **run_kwargs
    )
    parts = np.stack(
        [np.asarray(r["outp"], dtype=np.float32) for r in res.results]
    )  # [8, BH, N] f32
    out = parts.sum(axis=0)  # [BH, N]
    out = out.reshape(B, DH, N).transpose(0, 2, 1)  # [B, N, DH]
    return np.ascontiguousarray(out.astype(np.float32)), res


def kernel(inputs, W_xes, b_xes, inci, w, b):
    out, _ = _run(inputs, W_xes, b_xes, inci, w, b)
    return out
